# revision 20
# baseline (speedup 1.0000x reference)
"""Trainium2 Bass kernel for a GQA attention block (LuluAttention).

Problem: hidden_states [2, 2048, 2048], 16 q heads / 4 kv heads of dim 128,
RoPE, softmax attention, output projection.

Sharding: 8 cores = 2 (batch) x 4 (query-row blocks of 512 rows).
Each core computes the full K/V for its batch (all 4 kv heads), Q for its
512-row query slice (all 16 heads), RoPE, attention, and the output
projection for its row slice. The full output is assembled on the host by
pure concatenation (no collectives needed).

All matmul operands are bf16 (1 PE cycle/row vs 4 for fp32); PSUM
accumulation stays fp32 and the final output is fp32. hs^T is kept
resident in SBUF (16 tiles of [128, 2048] bf16) so K and V read it from
SBUF instead of re-streaming HBM. Device-side layout is transposed
([head_dim, seq] with head_dim on SBUF partitions) throughout:
  - QT/KT come straight out of matmul(lhsT=W_slice, rhs=hsT)
  - scores are computed transposed: scoresT = K @ Q^T
  - exp(scoresT) (bf16) feeds the AV matmul directly (lhsT = V tile)
  - softmax denominator = ones128 @ expT (broadcast across partitions)
  - ctxT slices are directly the lhsT for the output projection
so no on-device transposes are needed anywhere.
"""

import os
import sys

if "/opt/trn_rl_repo" not in sys.path:
    sys.path.insert(0, "/opt/trn_rl_repo")

import numpy as np

B, S, H = 2, 2048, 2048
NH, NKV, D = 16, 4, 128
SQ = 512          # query rows per core
NCORES = 8
P = 128
NT = H // P       # 16 contraction tiles over hidden dim
ROPE_THETA = 10000.0
SCALE = 1.0 / float(np.sqrt(D))


def _rope_tables_T():
    """cosT/ssinT [D, S]: transposed RoPE tables with the rotate-half sign
    folded into ssin (negative for d<64)."""
    inv_freq = 1.0 / (ROPE_THETA ** (np.arange(0, D, 2, dtype=np.float64) / D))
    t = np.arange(S, dtype=np.float64)
    freqs = np.outer(t, inv_freq)                     # [S, D/2]
    emb = np.concatenate([freqs, freqs], axis=-1)     # [S, D]
    cos = np.cos(emb).astype(np.float32)
    sin = np.sin(emb).astype(np.float32)
    ssin = sin.copy()
    ssin[:, : D // 2] *= -1.0
    return np.ascontiguousarray(cos.T), np.ascontiguousarray(ssin.T)


def _build_program():
    from concourse import bacc, mybir, tile

    F32 = mybir.dt.float32
    BF16 = mybir.dt.bfloat16
    AF = mybir.ActivationFunctionType

    nc = bacc.Bacc(
        "TRN2", target_bir_lowering=False, debug=False, num_devices=NCORES
    )

    # Inputs are packed into few tensors — per-buffer dispatch marshaling
    # costs ~50us/iter through the axon PJRT path.
    #   wkv [H+1, 1024]: cols 0:512 Wk, 512:1024 Wv; row H cols 512:1024 bv
    #   bqk [D, 20]:     cols 0:16 bq^T, 16:20 bk^T
    #   tbl [D, 5120]:   cosk | ssink | cosq | ssinq
    hsT = nc.dram_tensor("hsT", [H, S], BF16, kind="ExternalInput").ap()
    hsQ = nc.dram_tensor("hsQ", [H, SQ], BF16, kind="ExternalInput").ap()
    wq = nc.dram_tensor("wq", [H, NH * D], BF16, kind="ExternalInput").ap()
    wkv = nc.dram_tensor(
        "wkv", [H + 1, 2 * NKV * D], BF16, kind="ExternalInput"
    ).ap()
    wo = nc.dram_tensor("wo", [NH * D, H], BF16, kind="ExternalInput").ap()
    bqk = nc.dram_tensor("bqk", [D, NH + NKV], F32, kind="ExternalInput").ap()
    tbl_d = nc.dram_tensor(
        "tbl", [D, 2 * S + 2 * SQ], F32, kind="ExternalInput"
    ).ap()
    out = nc.dram_tensor("out", [SQ, H], F32, kind="ExternalOutput").ap()

    with tile.TileContext(nc) as tc:
        # ---- long-lived pools ----
        # left: constants + phase-2 operands; right: phase-scoped scratch
        # (stack discipline — release in LIFO order).
        cst = tc.alloc_tile_pool(name="cst", bufs=1)
        big = tc.alloc_tile_pool(name="big", bufs=1)

        ones1b = cst.tile([1, P], BF16, tag="ones1")
        nc.gpsimd.memset(ones1b[:], 1.0)
        ones128b = cst.tile([P, P], BF16, tag="ones128")
        nc.gpsimd.memset(ones128b[:], 1.0)
        # Constant tiles are allocated here but their loads are issued inside
        # the K loop (on the SP ring, between weight-tile loads) so nothing
        # delays the first matmul's operands.
        bqk_sb = cst.tile([D, NH + NKV], F32, tag="bqk")
        bvb_sb = cst.tile([1, NKV * D], BF16, tag="bvb")
        cosq_sb = cst.tile([D, SQ], F32, tag="cosq")
        ssinq_sb = cst.tile([D, SQ], F32, tag="ssinq")

        kt = [big.tile([D, S], BF16, tag=f"kt{g}", name=f"kt{g}")
              for g in range(NKV)]
        vt = [big.tile([P, NKV * D], BF16, tag=f"v{t}", name=f"v{t}")
              for t in range(S // P)]
        qt = [big.tile([D, SQ], BF16, tag=f"qt{h}", name=f"qt{h}")
              for h in range(NH)]
        ctx = [big.tile([D, SQ], BF16, tag=f"ctx{h}", name=f"ctx{h}")
               for h in range(NH)]

        # right stack: RoPE tables, then resident hs^T / hsQ^T.
        # hs/hq loads go on the Activation HWDGE ring so the SP ring is free
        # for the weight streams (DMA transfer time serializes per-ring).
        tbl = tc.alloc_tile_pool(name="tbl", bufs=1, side="right")
        cosk_sb = tbl.tile([D, S], F32, tag="cosk")
        ssink_sb = tbl.tile([D, S], F32, tag="ssink")

        hsp = tc.alloc_tile_pool(name="hsp", bufs=1, side="right")
        hs = [hsp.tile([P, S], BF16, tag=f"hs{ht}", name=f"hs{ht}")
              for ht in range(NT)]
        for ht in range(NT):
            nc.scalar.dma_start(hs[ht][:], hsT[ht * P : (ht + 1) * P, :])

        psK = tc.alloc_tile_pool(name="ps_k", bufs=8, space="PSUM")

        def rope_into(dst, src, tbl_cos, tbl_sin, pool, width):
            """dst (bf16 slice) = rope(src [128, width] f32)."""
            sh = pool.tile([P, 512], F32, tag="sh", bufs=2, name="sh")
            nc.scalar.dma_start(sh[0 : D // 2, :width], src[D // 2 : D, :width])
            nc.scalar.dma_start(sh[D // 2 : D, :width], src[0 : D // 2, :width])
            t1 = pool.tile([P, 512], F32, tag="rt1", bufs=2, name="rt1")
            nc.vector.tensor_mul(t1[:, :width], sh[:, :width], tbl_sin)
            t2 = pool.tile([P, 512], F32, tag="rt2", bufs=2, name="rt2")
            nc.vector.tensor_mul(t2[:, :width], src[:, :width], tbl_cos)
            nc.vector.tensor_add(dst, t1[:, :width], t2[:, :width])

        # ================= stage K =================
        # KT[g] [d=128, s2=2048] = (hs @ Wk + bk)^T, rope'd, bf16.
        s1p = tc.alloc_tile_pool(name="s1p", bufs=2, side="right")
        for sweep in range(2):
            gs = (2 * sweep, 2 * sweep + 1)
            banks = {
                (g, c): psK.tile([P, 512], F32, tag="pj", name=f"pk_{g}_{c}")
                for g in gs
                for c in range(4)
            }
            for ht in range(NT):
                wkt = s1p.tile([P, 2 * D], BF16, tag="wk", bufs=3, name="wkt")
                nc.sync.dma_start(
                    wkt[:],
                    wkv[ht * P : (ht + 1) * P, gs[0] * D : (gs[1] + 1) * D],
                )
                if sweep == 0:
                    # slip the constant loads in between weight tiles
                    if ht == 1:
                        nc.sync.dma_start(cosk_sb[:], tbl_d[:, 0:S])
                    elif ht == 2:
                        nc.sync.dma_start(ssink_sb[:], tbl_d[:, S : 2 * S])
                    elif ht == 3:
                        nc.sync.dma_start(bqk_sb[:], bqk[:, :])
                    elif ht == 4:
                        nc.sync.dma_start(
                            bvb_sb[:], wkv[H : H + 1, NKV * D : 2 * NKV * D]
                        )
                    elif ht == 5:
                        nc.sync.dma_start(
                            cosq_sb[:], tbl_d[:, 2 * S : 2 * S + SQ]
                        )
                    elif ht == 6:
                        nc.sync.dma_start(
                            ssinq_sb[:],
                            tbl_d[:, 2 * S + SQ : 2 * S + 2 * SQ],
                        )
                for gi, g in enumerate(gs):
                    for c in range(4):
                        nc.tensor.matmul(
                            banks[(g, c)][:],
                            wkt[:, gi * D : (gi + 1) * D],
                            hs[ht][:, c * 512 : (c + 1) * 512],
                            start=(ht == 0),
                            stop=(ht == NT - 1),
                        )
            # All 8 bias copies first (frees the PSUM banks for the next
            # sweep quickly), then the rope chains trail on ACT/DVE.
            tmps = {}
            for g in gs:
                for c in range(4):
                    tmp = s1p.tile([P, 512], F32, tag="ktmp", bufs=8,
                                   name="ktmp")
                    nc.scalar.activation(
                        tmp[:], banks[(g, c)][:], AF.Identity,
                        bias=bqk_sb[:, NH + g : NH + g + 1],
                    )
                    tmps[(g, c)] = tmp
            for g in gs:
                for c in range(4):
                    rope_into(
                        kt[g][:, c * 512 : (c + 1) * 512], tmps[(g, c)][:],
                        cosk_sb[:, c * 512 : (c + 1) * 512],
                        ssink_sb[:, c * 512 : (c + 1) * 512],
                        s1p, 512,
                    )
        
        # ================= stage Q =================
        # QT[h] [d=128, s1=512] = (hs_q @ Wq + bq)^T, rope'd, bf16.
        # Emitted before stage V so Q's RoPE (ACT/DVE) overlaps V's matmuls.
        for sweep in range(2):
            hset = range(8 * sweep, 8 * sweep + 8)
            qbank = {
                h: psK.tile([P, SQ], F32, tag="pj", name=f"pq_{h}")
                for h in hset
            }
            for ht in range(NT):
                hqt = s1p.tile([P, SQ], BF16, tag="hq", bufs=4, name="hqt")
                nc.sync.dma_start(hqt[:], hsQ[ht * P : (ht + 1) * P, :])
                wqt = s1p.tile([P, 8 * D], BF16, tag="wq", bufs=4, name="wqt")
                nc.sync.dma_start(
                    wqt[:],
                    wq[ht * P : (ht + 1) * P,
                       sweep * 8 * D : (sweep + 1) * 8 * D],
                )
                for i, h in enumerate(hset):
                    nc.tensor.matmul(
                        qbank[h][:],
                        wqt[:, i * D : (i + 1) * D],
                        hqt[:],
                        start=(ht == 0),
                        stop=(ht == NT - 1),
                    )
            qtmps = {}
            for h in hset:
                tmp = s1p.tile([P, 512], F32, tag="ktmp", bufs=8, name="qtmp")
                nc.scalar.activation(
                    tmp[:], qbank[h][:], AF.Identity,
                    bias=bqk_sb[:, h : h + 1],
                )
                qtmps[h] = tmp
            for h in hset:
                rope_into(qt[h][:], qtmps[h][:], cosq_sb[:], ssinq_sb[:],
                          s1p, SQ)
        
        # ================= stage V =================
        # V[t] [s2-tile=128, 4*128] = hs @ Wv + bv (natural layout), bf16.
        for sweep in range(2):
            ts = range(8 * sweep, 8 * sweep + 8)
            vbank = {
                t: psK.tile([P, NKV * D], F32, tag="pj", name=f"pv_{t}")
                for t in ts
            }
            for ht in range(NT):
                wvt = s1p.tile([P, NKV * D], BF16, tag="wv", bufs=3, name="wvt")
                nc.sync.dma_start(
                    wvt[:],
                    wkv[ht * P : (ht + 1) * P, NKV * D : 2 * NKV * D],
                )
                for t in ts:
                    nc.tensor.matmul(
                        vbank[t][:],
                        hs[ht][:, t * P : (t + 1) * P],
                        wvt[:],
                        start=(ht == 0),
                        stop=False,
                    )
            for t in ts:
                nc.tensor.matmul(
                    vbank[t][:], ones1b[:], bvb_sb[:], start=False, stop=True
                )
                nc.scalar.copy(vt[t][:], vbank[t][:])
        s1p.release()
        hsp.release()
        tbl.release()
        psK.release()

        # ================= phase 2: attention =================
        pa = tc.alloc_tile_pool(name="ps_a", bufs=5, space="PSUM")
        pc = tc.alloc_tile_pool(name="ps_c", bufs=3, space="PSUM")
        asp = tc.alloc_tile_pool(name="asp", bufs=2, side="right")
        NTT = S // P  # 16 key tiles

        for h in range(NH):
            g = h // (NH // NKV)
            ctx_ps = pc.tile([P, SQ], F32, tag="acc", name=f"ctxps{h}")
            den_ps = pc.tile([P, SQ], F32, tag="acc", name=f"denps{h}")
            at = {}

            def emit_sc(t):
                sc = pa.tile([P, SQ], F32, tag="sc", bufs=5, name="sc")
                nc.tensor.matmul(
                    sc[:],
                    kt[g][:, t * P : (t + 1) * P],
                    qt[h][:],
                    start=True,
                    stop=True,
                )
                a = asp.tile([P, SQ], BF16, tag="at", bufs=5, name="at")
                nc.scalar.activation(a[:], sc[:], AF.Exp, scale=SCALE)
                at[t] = a

            emit_sc(0)
            emit_sc(1)
            for t in range(NTT):
                if t + 2 < NTT:
                    emit_sc(t + 2)
                nc.tensor.matmul(
                    ctx_ps[:],
                    vt[t][:, g * D : (g + 1) * D],
                    at[t][:],
                    start=(t == 0),
                    stop=(t == NTT - 1),
                )
                nc.tensor.matmul(
                    den_ps[:],
                    ones128b[:],
                    at[t][:],
                    start=(t == 0),
                    stop=(t == NTT - 1),
                )
                del at[t]
            rc = asp.tile([P, SQ], F32, tag="rc", bufs=2, name="rc")
            nc.vector.reciprocal(rc[:], den_ps[:])
            nc.vector.tensor_mul(ctx[h][:], ctx_ps[:], rc[:])
        asp.release()
        pc.release()
        pa.release()

        # ================= phase 3: output projection =================
        po = tc.alloc_tile_pool(name="ps_o", bufs=8, space="PSUM")
        wso = tc.alloc_tile_pool(name="wso", bufs=4, side="right")
        for hc in range(4):
            banks = [
                po.tile([P, 512], F32, tag="po", name=f"po_{hc}_{i}")
                for i in range(4)
            ]
            for h in range(NH):
                wot = wso.tile([P, 512], BF16, tag="wo", bufs=4, name="wot")
                nc.sync.dma_start(
                    wot[:],
                    wo[h * D : (h + 1) * D, hc * 512 : (hc + 1) * 512],
                )
                for s1t in range(4):
                    nc.tensor.matmul(
                        banks[s1t][:],
                        ctx[h][:, s1t * P : (s1t + 1) * P],
                        wot[:],
                        start=(h == 0),
                        stop=(h == NH - 1),
                    )
            for s1t in range(4):
                ob = wso.tile([P, 512], F32, tag="ob", bufs=4, name="ob")
                nc.scalar.copy(ob[:], banks[s1t][:])
                nc.scalar.dma_start(
                    out[s1t * P : (s1t + 1) * P, hc * 512 : (hc + 1) * 512],
                    ob[:],
                )
        wso.release()
        po.release()
        big.release()
        cst.release()

    nc.compile()
    return nc


_PROGRAM_CACHE = {}


def _get_program():
    if "nc" not in _PROGRAM_CACHE:
        _PROGRAM_CACHE["nc"] = _build_program()
    return _PROGRAM_CACHE["nc"]


def _prepare_in_maps(hidden_states, Wq, bq, Wk, bk, Wv, bv, Wo):
    import ml_dtypes

    BF = ml_dtypes.bfloat16

    hidden_states = np.asarray(hidden_states, dtype=np.float32)
    Wq = np.asarray(Wq, dtype=np.float32)
    bq = np.asarray(bq, dtype=np.float32)
    Wk = np.asarray(Wk, dtype=np.float32)
    bk = np.asarray(bk, dtype=np.float32)
    Wv = np.asarray(Wv, dtype=np.float32)
    bv = np.asarray(bv, dtype=np.float32)
    Wo = np.asarray(Wo, dtype=np.float32)

    cosT, ssinT = _rope_tables_T()
    bqk_h = np.ascontiguousarray(
        np.concatenate(
            [bq.reshape(NH, D).T, bk.reshape(NKV, D).T], axis=1
        )
    )  # [128, 20]

    wkv_h = np.zeros((H + 1, 2 * NKV * D), dtype=BF)
    wkv_h[:H, : NKV * D] = Wk.astype(BF)
    wkv_h[:H, NKV * D :] = Wv.astype(BF)
    wkv_h[H, NKV * D :] = bv.astype(BF)

    wq_h = Wq.astype(BF)
    wo_h = Wo.astype(BF)
    hsT_b = [np.ascontiguousarray(hidden_states[b].T).astype(BF)
             for b in range(B)]

    in_maps = []
    for core in range(NCORES):
        b, tq = core // 4, core % 4
        qoff = tq * SQ
        tbl_h = np.concatenate(
            [cosT, ssinT,
             cosT[:, qoff : qoff + SQ], ssinT[:, qoff : qoff + SQ]],
            axis=1,
        )
        in_maps.append(
            {
                "hsT": hsT_b[b],
                "hsQ": np.ascontiguousarray(hsT_b[b][:, qoff : qoff + SQ]),
                "wq": wq_h,
                "wkv": wkv_h,
                "wo": wo_h,
                "bqk": bqk_h,
                "tbl": np.ascontiguousarray(tbl_h),
            }
        )
    return in_maps


def kernel(hidden_states, Wq, bq, Wk, bk, Wv, bv, Wo):
    from concourse.bass_utils import run_bass_kernel_spmd

    in_maps = _prepare_in_maps(hidden_states, Wq, bq, Wk, bk, Wv, bv, Wo)
    nc = _get_program()
    res = run_bass_kernel_spmd(
        nc, in_maps, core_ids=list(range(NCORES)), trace=False
    )

    out_full = np.empty((B, S, H), dtype=np.float32)
    for core in range(NCORES):
        b, tq = core // 4, core % 4
        out_full[b, tq * SQ : (tq + 1) * SQ, :] = res.results[core]["out"]
    return out_full


# revision 25
# speedup vs baseline: 3.6785x; 3.6785x over previous
"""Trainium2 Bass kernel for a GQA attention block (LuluAttention).

Problem: hidden_states [2, 2048, 2048], 16 q heads / 4 kv heads of dim 128,
RoPE, softmax attention, output projection.

Sharding: 8 cores = 2 (batch) x 4 (query-row blocks of 512 rows).
Each core computes the full K/V for its batch (all 4 kv heads), Q for its
512-row query slice (all 16 heads), RoPE, attention, and the output
projection for its row slice. The full output is assembled on the host by
pure concatenation (no collectives needed).

All matmul operands are bf16 (1 PE cycle/row vs 4 for fp32); PSUM
accumulation stays fp32 and the final output is fp32. hs^T is kept
resident in SBUF (16 tiles of [128, 2048] bf16) so K and V read it from
SBUF instead of re-streaming HBM. Device-side layout is transposed
([head_dim, seq] with head_dim on SBUF partitions) throughout:
  - QT/KT come straight out of matmul(lhsT=W_slice, rhs=hsT)
  - scores are computed transposed: scoresT = K @ Q^T
  - exp(scoresT) (bf16) feeds the AV matmul directly (lhsT = V tile)
  - softmax denominator = ones128 @ expT (broadcast across partitions)
  - ctxT slices are directly the lhsT for the output projection
so no on-device transposes are needed anywhere.
"""

import os
import sys

if "/opt/trn_rl_repo" not in sys.path:
    sys.path.insert(0, "/opt/trn_rl_repo")

import numpy as np

B, S, H = 2, 2048, 2048
NH, NKV, D = 16, 4, 128
SQ = 512          # query rows per core
NCORES = 8
P = 128
NT = H // P       # 16 contraction tiles over hidden dim
ROPE_THETA = 10000.0
SCALE = 1.0 / float(np.sqrt(D))


def _rope_tables_T():
    """cosT/ssinT [D, S]: transposed RoPE tables with the rotate-half sign
    folded into ssin (negative for d<64)."""
    inv_freq = 1.0 / (ROPE_THETA ** (np.arange(0, D, 2, dtype=np.float64) / D))
    t = np.arange(S, dtype=np.float64)
    freqs = np.outer(t, inv_freq)                     # [S, D/2]
    emb = np.concatenate([freqs, freqs], axis=-1)     # [S, D]
    cos = np.cos(emb).astype(np.float32)
    sin = np.sin(emb).astype(np.float32)
    ssin = sin.copy()
    ssin[:, : D // 2] *= -1.0
    return np.ascontiguousarray(cos.T), np.ascontiguousarray(ssin.T)


def _build_program():
    from concourse import bacc, mybir, tile

    F32 = mybir.dt.float32
    BF16 = mybir.dt.bfloat16
    AF = mybir.ActivationFunctionType

    nc = bacc.Bacc(
        "TRN2", target_bir_lowering=False, debug=False, num_devices=NCORES
    )

    # All inputs are packed into TWO tensors — per-buffer dispatch
    # marshaling costs real time per iteration through the axon PJRT path.
    # Each core's hs^T arrives column-ROTATED so its own query slice sits at
    # columns 0:SQ (attention is invariant to key order; the RoPE tables in
    # `misc` are rotated identically on the host).
    #   mega bf16 [8193, 2048]:
    #     rows     0:2048  hs^T (rotated)      rows 2048:4096  Wq
    #     rows  4096:6144  Wo                  rows 6144:8192  [Wk | Wv]
    #     row   8192       cols 512:1024 = bv
    #   misc f32 [128, 5140]: bq^T|bk^T (0:20), cosk (20:2068),
    #     ssink (2068:4116), cosq (4116:4628), ssinq (4628:5140)
    MG_HS, MG_WQ, MG_WO, MG_WKV, MG_BV = 0, H, 2 * H, 3 * H, 4 * H + 1
    mega = nc.dram_tensor(
        "mega", [4 * H + 1, S], BF16, kind="ExternalInput"
    ).ap()
    misc = nc.dram_tensor(
        "misc", [D, NH + NKV + 2 * S + 2 * SQ], F32, kind="ExternalInput"
    ).ap()
    out = nc.dram_tensor("out", [SQ, H], F32, kind="ExternalOutput").ap()
    MC_B, MC_CK, MC_SK = 0, NH + NKV, NH + NKV + S
    MC_CQ, MC_SQ = NH + NKV + 2 * S, NH + NKV + 2 * S + SQ

    with tile.TileContext(nc) as tc:
        # ---- long-lived pools ----
        # left: constants + phase-2 operands; right: phase-scoped scratch
        # (stack discipline — release in LIFO order).
        cst = tc.alloc_tile_pool(name="cst", bufs=1)
        big = tc.alloc_tile_pool(name="big", bufs=1)

        ones1b = cst.tile([1, P], BF16, tag="ones1")
        nc.gpsimd.memset(ones1b[:], 1.0)
        ones128b = cst.tile([P, P], BF16, tag="ones128")
        nc.gpsimd.memset(ones128b[:], 1.0)
        # Constant tiles are allocated here but their loads are issued inside
        # the K loop (on the SP ring, between weight-tile loads) so nothing
        # delays the first matmul's operands.
        bqk_sb = cst.tile([D, NH + NKV], F32, tag="bqk")
        bvb_sb = cst.tile([1, NKV * D], BF16, tag="bvb")
        cosq_sb = cst.tile([D, SQ], F32, tag="cosq")
        ssinq_sb = cst.tile([D, SQ], F32, tag="ssinq")

        kt = [big.tile([D, S], BF16, tag=f"kt{g}", name=f"kt{g}")
              for g in range(NKV)]
        vt = [big.tile([P, NKV * D], BF16, tag=f"v{t}", name=f"v{t}")
              for t in range(S // P)]
        qt = [big.tile([D, SQ], BF16, tag=f"qt{h}", name=f"qt{h}")
              for h in range(NH)]
        ctx = [big.tile([D, SQ], BF16, tag=f"ctx{h}", name=f"ctx{h}")
               for h in range(NH)]

        # right stack: RoPE tables, then resident hs^T / hsQ^T.
        # hs/hq loads go on the Activation HWDGE ring so the SP ring is free
        # for the weight streams (DMA transfer time serializes per-ring).
        tbl = tc.alloc_tile_pool(name="tbl", bufs=1, side="right")
        cosk_sb = tbl.tile([D, S], F32, tag="cosk")
        ssink_sb = tbl.tile([D, S], F32, tag="ssink")

        hsp = tc.alloc_tile_pool(name="hsp", bufs=1, side="right")
        hs = [hsp.tile([P, S], BF16, tag=f"hs{ht}", name=f"hs{ht}")
              for ht in range(NT)]
        for ht in range(NT):
            nc.scalar.dma_start(hs[ht][:], mega[ht * P : (ht + 1) * P, :])

        psK = tc.alloc_tile_pool(name="ps_k", bufs=8, space="PSUM")

        def rope_into(dst, src, tbl_cos, tbl_sin, pool, width):
            """dst (bf16 slice) = rope(src [128, width] f32)."""
            sh = pool.tile([P, 512], F32, tag="sh", bufs=2, name="sh")
            nc.scalar.dma_start(sh[0 : D // 2, :width], src[D // 2 : D, :width])
            nc.scalar.dma_start(sh[D // 2 : D, :width], src[0 : D // 2, :width])
            t1 = pool.tile([P, 512], F32, tag="rt1", bufs=2, name="rt1")
            nc.vector.tensor_mul(t1[:, :width], sh[:, :width], tbl_sin)
            t2 = pool.tile([P, 512], F32, tag="rt2", bufs=2, name="rt2")
            nc.vector.tensor_mul(t2[:, :width], src[:, :width], tbl_cos)
            nc.vector.tensor_add(dst, t1[:, :width], t2[:, :width])

        # ================= stage K =================
        # KT[g] [d=128, s2=2048] = (hs @ Wk + bk)^T, rope'd, bf16.
        s1p = tc.alloc_tile_pool(name="s1p", bufs=2, side="right")
        for sweep in range(2):
            gs = (2 * sweep, 2 * sweep + 1)
            banks = {
                (g, c): psK.tile([P, 512], F32, tag="pj", name=f"pk_{g}_{c}")
                for g in gs
                for c in range(4)
            }
            for ht in range(NT):
                wkt = s1p.tile([P, 2 * D], BF16, tag="wk", bufs=3, name="wkt")
                nc.sync.dma_start(
                    wkt[:],
                    mega[MG_WKV + ht * P : MG_WKV + (ht + 1) * P,
                         gs[0] * D : (gs[1] + 1) * D],
                )
                if sweep == 0:
                    # slip the constant loads in between weight tiles
                    if ht == 1:
                        nc.sync.dma_start(cosk_sb[:], misc[:, MC_CK : MC_CK + S])
                    elif ht == 2:
                        nc.sync.dma_start(ssink_sb[:], misc[:, MC_SK : MC_SK + S])
                    elif ht == 3:
                        nc.sync.dma_start(bqk_sb[:], misc[:, MC_B : MC_B + NH + NKV])
                    elif ht == 4:
                        nc.sync.dma_start(
                            bvb_sb[:],
                            mega[MG_BV - 1 : MG_BV, NKV * D : 2 * NKV * D],
                        )
                    elif ht == 5:
                        nc.sync.dma_start(
                            cosq_sb[:], misc[:, MC_CQ : MC_CQ + SQ]
                        )
                    elif ht == 6:
                        nc.sync.dma_start(
                            ssinq_sb[:], misc[:, MC_SQ : MC_SQ + SQ]
                        )
                for gi, g in enumerate(gs):
                    for c in range(4):
                        nc.tensor.matmul(
                            banks[(g, c)][:],
                            wkt[:, gi * D : (gi + 1) * D],
                            hs[ht][:, c * 512 : (c + 1) * 512],
                            start=(ht == 0),
                            stop=(ht == NT - 1),
                        )
            # All 8 bias copies first (frees the PSUM banks for the next
            # sweep quickly), then the rope chains trail on ACT/DVE.
            tmps = {}
            for g in gs:
                for c in range(4):
                    tmp = s1p.tile([P, 512], F32, tag="ktmp", bufs=8,
                                   name="ktmp")
                    nc.scalar.activation(
                        tmp[:], banks[(g, c)][:], AF.Identity,
                        bias=bqk_sb[:, NH + g : NH + g + 1],
                    )
                    tmps[(g, c)] = tmp
            for g in gs:
                for c in range(4):
                    rope_into(
                        kt[g][:, c * 512 : (c + 1) * 512], tmps[(g, c)][:],
                        cosk_sb[:, c * 512 : (c + 1) * 512],
                        ssink_sb[:, c * 512 : (c + 1) * 512],
                        s1p, 512,
                    )
        
        # ================= stage Q =================
        # QT[h] [d=128, s1=512] = (hs_q @ Wq + bq)^T, rope'd, bf16.
        # Emitted before stage V so Q's RoPE (ACT/DVE) overlaps V's matmuls.
        for sweep in range(2):
            hset = range(8 * sweep, 8 * sweep + 8)
            qbank = {
                h: psK.tile([P, SQ], F32, tag="pj", name=f"pq_{h}")
                for h in hset
            }
            for ht in range(NT):
                wqt = s1p.tile([P, 8 * D], BF16, tag="wq", bufs=4, name="wqt")
                nc.sync.dma_start(
                    wqt[:],
                    mega[MG_WQ + ht * P : MG_WQ + (ht + 1) * P,
                         sweep * 8 * D : (sweep + 1) * 8 * D],
                )
                for i, h in enumerate(hset):
                    nc.tensor.matmul(
                        qbank[h][:],
                        wqt[:, i * D : (i + 1) * D],
                        hs[ht][:, 0:SQ],
                        start=(ht == 0),
                        stop=(ht == NT - 1),
                    )
            qtmps = {}
            for h in hset:
                tmp = s1p.tile([P, 512], F32, tag="ktmp", bufs=8, name="qtmp")
                nc.scalar.activation(
                    tmp[:], qbank[h][:], AF.Identity,
                    bias=bqk_sb[:, h : h + 1],
                )
                qtmps[h] = tmp
            for h in hset:
                rope_into(qt[h][:], qtmps[h][:], cosq_sb[:], ssinq_sb[:],
                          s1p, SQ)
        
        # ================= stage V =================
        # V[t] [s2-tile=128, 4*128] = hs @ Wv + bv (natural layout), bf16.
        for sweep in range(2):
            ts = range(8 * sweep, 8 * sweep + 8)
            vbank = {
                t: psK.tile([P, NKV * D], F32, tag="pj", name=f"pv_{t}")
                for t in ts
            }
            for ht in range(NT):
                wvt = s1p.tile([P, NKV * D], BF16, tag="wv", bufs=3, name="wvt")
                nc.sync.dma_start(
                    wvt[:],
                    mega[MG_WKV + ht * P : MG_WKV + (ht + 1) * P,
                         NKV * D : 2 * NKV * D],
                )
                for t in ts:
                    nc.tensor.matmul(
                        vbank[t][:],
                        hs[ht][:, t * P : (t + 1) * P],
                        wvt[:],
                        start=(ht == 0),
                        stop=False,
                    )
            for t in ts:
                nc.tensor.matmul(
                    vbank[t][:], ones1b[:], bvb_sb[:], start=False, stop=True
                )
                nc.scalar.copy(vt[t][:], vbank[t][:])
        s1p.release()
        hsp.release()
        tbl.release()
        psK.release()

        # ================= phase 2: attention =================
        # wso is allocated before asp (deeper on the right stack) so the
        # output-projection weight prefetch can run on the idle SP ring
        # during attention.
        wso = tc.alloc_tile_pool(name="wso", bufs=4, side="right")
        pa = tc.alloc_tile_pool(name="ps_a", bufs=5, space="PSUM")
        pc = tc.alloc_tile_pool(name="ps_c", bufs=3, space="PSUM")
        asp = tc.alloc_tile_pool(name="asp", bufs=2, side="right")
        NTT = S // P  # 16 key tiles

        for h in range(NH):
            g = h // (NH // NKV)
            ctx_ps = pc.tile([P, SQ], F32, tag="acc", name=f"ctxps{h}")
            den_ps = pc.tile([P, SQ], F32, tag="acc", name=f"denps{h}")
            at = {}

            def emit_sc(t):
                sc = pa.tile([P, SQ], F32, tag="sc", bufs=5, name="sc")
                nc.tensor.matmul(
                    sc[:],
                    kt[g][:, t * P : (t + 1) * P],
                    qt[h][:],
                    start=True,
                    stop=True,
                )
                a = asp.tile([P, SQ], BF16, tag="at", bufs=5, name="at")
                nc.scalar.activation(a[:], sc[:], AF.Exp, scale=SCALE)
                at[t] = a

            emit_sc(0)
            emit_sc(1)
            for t in range(NTT):
                if t + 2 < NTT:
                    emit_sc(t + 2)
                nc.tensor.matmul(
                    ctx_ps[:],
                    vt[t][:, g * D : (g + 1) * D],
                    at[t][:],
                    start=(t == 0),
                    stop=(t == NTT - 1),
                )
                nc.tensor.matmul(
                    den_ps[:],
                    ones128b[:],
                    at[t][:],
                    start=(t == 0),
                    stop=(t == NTT - 1),
                )
                del at[t]
            rc = asp.tile([P, SQ], F32, tag="rc", bufs=2, name="rc")
            nc.vector.reciprocal(rc[:], den_ps[:])
            nc.vector.tensor_mul(ctx[h][:], ctx_ps[:], rc[:])
        asp.release()
        pc.release()
        pa.release()

        # ================= phase 3: output projection =================
        # s1t-major: each PSUM bank finishes all 16 head matmuls before the
        # next bank starts, so its copy+store overlaps the next bank's
        # matmuls instead of piling up at the end. The per-chunk Wo tiles
        # are prefetched (SP is idle during attention).
        po = tc.alloc_tile_pool(name="ps_o", bufs=8, space="PSUM")
        for hc in range(4):
            wots = []
            for h in range(NH):
                wot = wso.tile([P, 512], BF16, tag="wo", bufs=18, name="wot")
                nc.sync.dma_start(
                    wot[:],
                    mega[MG_WO + h * D : MG_WO + (h + 1) * D,
                         hc * 512 : (hc + 1) * 512],
                )
                wots.append(wot)
            banks = [
                po.tile([P, 512], F32, tag="po", name=f"po_{hc}_{i}")
                for i in range(4)
            ]
            for s1t in range(4):
                for h in range(NH):
                    nc.tensor.matmul(
                        banks[s1t][:],
                        ctx[h][:, s1t * P : (s1t + 1) * P],
                        wots[h][:],
                        start=(h == 0),
                        stop=(h == NH - 1),
                    )
                ob = wso.tile([P, 512], F32, tag="ob", bufs=4, name="ob")
                nc.scalar.copy(ob[:], banks[s1t][:])
                nc.scalar.dma_start(
                    out[s1t * P : (s1t + 1) * P, hc * 512 : (hc + 1) * 512],
                    ob[:],
                )
        wso.release()
        po.release()
        big.release()
        cst.release()

    nc.compile()
    return nc


_PROGRAM_CACHE = {}


def _get_program():
    if "nc" not in _PROGRAM_CACHE:
        _PROGRAM_CACHE["nc"] = _build_program()
    return _PROGRAM_CACHE["nc"]


def _prepare_in_maps(hidden_states, Wq, bq, Wk, bk, Wv, bv, Wo):
    import ml_dtypes

    BF = ml_dtypes.bfloat16

    hidden_states = np.asarray(hidden_states, dtype=np.float32)
    Wq = np.asarray(Wq, dtype=np.float32)
    bq = np.asarray(bq, dtype=np.float32)
    Wk = np.asarray(Wk, dtype=np.float32)
    bk = np.asarray(bk, dtype=np.float32)
    Wv = np.asarray(Wv, dtype=np.float32)
    bv = np.asarray(bv, dtype=np.float32)
    Wo = np.asarray(Wo, dtype=np.float32)

    cosT, ssinT = _rope_tables_T()
    bqk_h = np.concatenate(
        [bq.reshape(NH, D).T, bk.reshape(NKV, D).T], axis=1
    )  # [128, 20]

    # weight block of `mega` (rows H:) is identical for every core
    wblock = np.empty((3 * H + 1, H), dtype=BF)
    wblock[0:H] = Wq.astype(BF)
    wblock[H : 2 * H] = Wo.astype(BF)
    wblock[2 * H :] = 0
    wblock[2 * H : 3 * H, : NKV * D] = Wk.astype(BF)
    wblock[2 * H : 3 * H, NKV * D : 2 * NKV * D] = Wv.astype(BF)
    wblock[3 * H, NKV * D : 2 * NKV * D] = bv.astype(BF)

    hsT_b = [np.ascontiguousarray(hidden_states[b].T).astype(BF)
             for b in range(B)]

    def roll(a, qoff):
        if qoff == 0:
            return a
        return np.concatenate([a[:, qoff:], a[:, :qoff]], axis=1)

    in_maps = []
    for core in range(NCORES):
        b, tq = core // 4, core % 4
        qoff = tq * SQ
        mega_h = np.empty((4 * H + 1, H), dtype=BF)
        mega_h[0:H] = roll(hsT_b[b], qoff)
        mega_h[H:] = wblock
        cos_r, ssin_r = roll(cosT, qoff), roll(ssinT, qoff)
        misc_h = np.concatenate(
            [bqk_h, cos_r, ssin_r, cos_r[:, :SQ], ssin_r[:, :SQ]], axis=1
        )
        in_maps.append(
            {
                "mega": mega_h,
                "misc": np.ascontiguousarray(misc_h),
            }
        )
    return in_maps


def kernel(hidden_states, Wq, bq, Wk, bk, Wv, bv, Wo):
    from concourse.bass_utils import run_bass_kernel_spmd

    in_maps = _prepare_in_maps(hidden_states, Wq, bq, Wk, bk, Wv, bv, Wo)
    nc = _get_program()
    res = run_bass_kernel_spmd(
        nc, in_maps, core_ids=list(range(NCORES)), trace=False
    )

    out_full = np.empty((B, S, H), dtype=np.float32)
    for core in range(NCORES):
        b, tq = core // 4, core % 4
        out_full[b, tq * SQ : (tq + 1) * SQ, :] = res.results[core]["out"]
    return out_full


# revision 27
# speedup vs baseline: 4.5796x; 1.2450x over previous
"""Trainium2 Bass kernel for a GQA attention block (LuluAttention).

Problem: hidden_states [2, 2048, 2048], 16 q heads / 4 kv heads of dim 128,
RoPE, softmax attention, output projection.

Sharding: 8 cores = 2 (batch) x 4 (query-row blocks of 512 rows).
Each core computes the full K/V for its batch (all 4 kv heads), Q for its
512-row query slice (all 16 heads), RoPE, attention, and the output
projection for its row slice. The full output is assembled on the host by
pure concatenation (no collectives needed).

All matmul operands are bf16 (1 PE cycle/row vs 4 for fp32); PSUM
accumulation stays fp32 and the final output is fp32. hs^T is kept
resident in SBUF (16 tiles of [128, 2048] bf16) so K and V read it from
SBUF instead of re-streaming HBM. Device-side layout is transposed
([head_dim, seq] with head_dim on SBUF partitions) throughout:
  - QT/KT come straight out of matmul(lhsT=W_slice, rhs=hsT)
  - scores are computed transposed: scoresT = K @ Q^T
  - exp(scoresT) (bf16) feeds the AV matmul directly (lhsT = V tile)
  - softmax denominator = ones128 @ expT (broadcast across partitions)
  - ctxT slices are directly the lhsT for the output projection
so no on-device transposes are needed anywhere.
"""

import os
import sys

if "/opt/trn_rl_repo" not in sys.path:
    sys.path.insert(0, "/opt/trn_rl_repo")

import numpy as np

B, S, H = 2, 2048, 2048
NH, NKV, D = 16, 4, 128
SQ = 512          # query rows per core
NCORES = 8
P = 128
NT = H // P       # 16 contraction tiles over hidden dim
ROPE_THETA = 10000.0
SCALE = 1.0 / float(np.sqrt(D))


def _rope_tables_T():
    """cosT/ssinT [D, S]: transposed RoPE tables with the rotate-half sign
    folded into ssin (negative for d<64)."""
    inv_freq = 1.0 / (ROPE_THETA ** (np.arange(0, D, 2, dtype=np.float64) / D))
    t = np.arange(S, dtype=np.float64)
    freqs = np.outer(t, inv_freq)                     # [S, D/2]
    emb = np.concatenate([freqs, freqs], axis=-1)     # [S, D]
    cos = np.cos(emb).astype(np.float32)
    sin = np.sin(emb).astype(np.float32)
    ssin = sin.copy()
    ssin[:, : D // 2] *= -1.0
    return np.ascontiguousarray(cos.T), np.ascontiguousarray(ssin.T)


def _build_program():
    from concourse import bacc, mybir, tile

    F32 = mybir.dt.float32
    BF16 = mybir.dt.bfloat16
    AF = mybir.ActivationFunctionType

    nc = bacc.Bacc(
        "TRN2", target_bir_lowering=False, debug=False, num_devices=NCORES
    )

    # All inputs are packed into TWO tensors — per-buffer dispatch
    # marshaling costs real time per iteration through the axon PJRT path.
    # Each core's hs^T arrives column-ROTATED so its own query slice sits at
    # columns 0:SQ (attention is invariant to key order; the RoPE tables in
    # `misc` are rotated identically on the host).
    #   mega bf16 [8193, 2048]:
    #     rows     0:2048  hs^T (rotated)      rows 2048:4096  Wq
    #     rows  4096:6144  Wo                  rows 6144:8192  [Wk | Wv]
    #     row   8192       cols 512:1024 = bv
    #   misc f32 [128, 5140]: bq^T|bk^T (0:20), cosk (20:2068),
    #     ssink (2068:4116), cosq (4116:4628), ssinq (4628:5140)
    MG_HS, MG_WQ, MG_WO, MG_WKV, MG_BV = 0, H, 2 * H, 3 * H, 4 * H + 1
    mega = nc.dram_tensor(
        "mega", [4 * H + 1, S], BF16, kind="ExternalInput"
    ).ap()
    misc = nc.dram_tensor(
        "misc", [D, NH + NKV + 2 * S + 2 * SQ], F32, kind="ExternalInput"
    ).ap()
    out = nc.dram_tensor("out", [SQ, H], F32, kind="ExternalOutput").ap()
    MC_B, MC_CK, MC_SK = 0, NH + NKV, NH + NKV + S
    MC_CQ, MC_SQ = NH + NKV + 2 * S, NH + NKV + 2 * S + SQ

    with tile.TileContext(nc) as tc:
        # ---- long-lived pools ----
        # left: constants + phase-2 operands; right: phase-scoped scratch
        # (stack discipline — release in LIFO order).
        cst = tc.alloc_tile_pool(name="cst", bufs=1)
        big = tc.alloc_tile_pool(name="big", bufs=1)

        ones1b = cst.tile([1, P], BF16, tag="ones1")
        nc.gpsimd.memset(ones1b[:], 1.0)
        ones128b = cst.tile([P, P], BF16, tag="ones128")
        nc.gpsimd.memset(ones128b[:], 1.0)
        # Constant tiles are allocated here but their loads are issued inside
        # the K loop (on the SP ring, between weight-tile loads) so nothing
        # delays the first matmul's operands.
        bqk_sb = cst.tile([D, NH + NKV], F32, tag="bqk")
        bvb_sb = cst.tile([1, NKV * D], BF16, tag="bvb")
        cosq_sb = cst.tile([D, SQ], F32, tag="cosq")
        ssinq_sb = cst.tile([D, SQ], F32, tag="ssinq")

        kt = [big.tile([D, S], BF16, tag=f"kt{g}", name=f"kt{g}")
              for g in range(NKV)]
        vt = [big.tile([P, NKV * D], BF16, tag=f"v{t}", name=f"v{t}")
              for t in range(S // P)]
        qt = [big.tile([D, SQ], BF16, tag=f"qt{h}", name=f"qt{h}")
              for h in range(NH)]
        ctx = [big.tile([D, SQ], BF16, tag=f"ctx{h}", name=f"ctx{h}")
               for h in range(NH)]

        # right stack: RoPE tables, then resident hs^T / hsQ^T.
        # hs/hq loads go on the Activation HWDGE ring so the SP ring is free
        # for the weight streams (DMA transfer time serializes per-ring).
        tbl = tc.alloc_tile_pool(name="tbl", bufs=1, side="right")
        cosk_sb = tbl.tile([D, S], F32, tag="cosk")
        ssink_sb = tbl.tile([D, S], F32, tag="ssink")

        hsp = tc.alloc_tile_pool(name="hsp", bufs=1, side="right")
        hs = [hsp.tile([P, S], BF16, tag=f"hs{ht}", name=f"hs{ht}")
              for ht in range(NT)]
        # hs[0] goes first on the otherwise-empty SP ring so the first K
        # matmul starts ~2us earlier; the rest stream on the ACT ring.
        nc.sync.dma_start(hs[0][:], mega[0:P, :])
        for ht in range(1, NT):
            nc.scalar.dma_start(hs[ht][:], mega[ht * P : (ht + 1) * P, :])

        psK = tc.alloc_tile_pool(name="ps_k", bufs=8, space="PSUM")

        def rope_into(dst, src, tbl_cos, tbl_sin, pool, width):
            """dst (bf16 slice) = rope(src [128, width] f32)."""
            sh = pool.tile([P, 512], F32, tag="sh", bufs=2, name="sh")
            nc.scalar.dma_start(sh[0 : D // 2, :width], src[D // 2 : D, :width])
            nc.scalar.dma_start(sh[D // 2 : D, :width], src[0 : D // 2, :width])
            t1 = pool.tile([P, 512], F32, tag="rt1", bufs=2, name="rt1")
            nc.vector.tensor_mul(t1[:, :width], sh[:, :width], tbl_sin)
            t2 = pool.tile([P, 512], F32, tag="rt2", bufs=2, name="rt2")
            nc.vector.tensor_mul(t2[:, :width], src[:, :width], tbl_cos)
            nc.vector.tensor_add(dst, t1[:, :width], t2[:, :width])

        # ================= stage K =================
        # KT[g] [d=128, s2=2048] = (hs @ Wk + bk)^T, rope'd, bf16.
        s1p = tc.alloc_tile_pool(name="s1p", bufs=2, side="right")
        for sweep in range(2):
            gs = (2 * sweep, 2 * sweep + 1)
            banks = {
                (g, c): psK.tile([P, 512], F32, tag="pj", name=f"pk_{g}_{c}")
                for g in gs
                for c in range(4)
            }
            for ht in range(NT):
                wkt = s1p.tile([P, 2 * D], BF16, tag="wk", bufs=3, name="wkt")
                nc.sync.dma_start(
                    wkt[:],
                    mega[MG_WKV + ht * P : MG_WKV + (ht + 1) * P,
                         gs[0] * D : (gs[1] + 1) * D],
                )
                if sweep == 0:
                    # slip the constant loads in between weight tiles
                    if ht == 1:
                        nc.sync.dma_start(cosk_sb[:], misc[:, MC_CK : MC_CK + S])
                    elif ht == 2:
                        nc.sync.dma_start(ssink_sb[:], misc[:, MC_SK : MC_SK + S])
                    elif ht == 3:
                        nc.sync.dma_start(bqk_sb[:], misc[:, MC_B : MC_B + NH + NKV])
                    elif ht == 4:
                        nc.sync.dma_start(
                            bvb_sb[:],
                            mega[MG_BV - 1 : MG_BV, NKV * D : 2 * NKV * D],
                        )
                    elif ht == 5:
                        nc.sync.dma_start(
                            cosq_sb[:], misc[:, MC_CQ : MC_CQ + SQ]
                        )
                    elif ht == 6:
                        nc.sync.dma_start(
                            ssinq_sb[:], misc[:, MC_SQ : MC_SQ + SQ]
                        )
                for gi, g in enumerate(gs):
                    for c in range(4):
                        nc.tensor.matmul(
                            banks[(g, c)][:],
                            wkt[:, gi * D : (gi + 1) * D],
                            hs[ht][:, c * 512 : (c + 1) * 512],
                            start=(ht == 0),
                            stop=(ht == NT - 1),
                        )
            # All 8 bias copies first (frees the PSUM banks for the next
            # sweep quickly), then the rope chains trail on ACT/DVE.
            tmps = {}
            for g in gs:
                for c in range(4):
                    tmp = s1p.tile([P, 512], F32, tag="ktmp", bufs=8,
                                   name="ktmp")
                    nc.scalar.activation(
                        tmp[:], banks[(g, c)][:], AF.Identity,
                        bias=bqk_sb[:, NH + g : NH + g + 1],
                    )
                    tmps[(g, c)] = tmp
            for g in gs:
                for c in range(4):
                    rope_into(
                        kt[g][:, c * 512 : (c + 1) * 512], tmps[(g, c)][:],
                        cosk_sb[:, c * 512 : (c + 1) * 512],
                        ssink_sb[:, c * 512 : (c + 1) * 512],
                        s1p, 512,
                    )
        
        # ================= stage Q =================
        # QT[h] [d=128, s1=512] = (hs_q @ Wq + bq)^T, rope'd, bf16.
        # Emitted before stage V so Q's RoPE (ACT/DVE) overlaps V's matmuls.
        for sweep in range(2):
            hset = range(8 * sweep, 8 * sweep + 8)
            qbank = {
                h: psK.tile([P, SQ], F32, tag="pj", name=f"pq_{h}")
                for h in hset
            }
            for ht in range(NT):
                wqt = s1p.tile([P, 8 * D], BF16, tag="wq", bufs=4, name="wqt")
                nc.sync.dma_start(
                    wqt[:],
                    mega[MG_WQ + ht * P : MG_WQ + (ht + 1) * P,
                         sweep * 8 * D : (sweep + 1) * 8 * D],
                )
                for i, h in enumerate(hset):
                    nc.tensor.matmul(
                        qbank[h][:],
                        wqt[:, i * D : (i + 1) * D],
                        hs[ht][:, 0:SQ],
                        start=(ht == 0),
                        stop=(ht == NT - 1),
                    )
            qtmps = {}
            for h in hset:
                tmp = s1p.tile([P, 512], F32, tag="ktmp", bufs=8, name="qtmp")
                nc.scalar.activation(
                    tmp[:], qbank[h][:], AF.Identity,
                    bias=bqk_sb[:, h : h + 1],
                )
                qtmps[h] = tmp
            for h in hset:
                rope_into(qt[h][:], qtmps[h][:], cosq_sb[:], ssinq_sb[:],
                          s1p, SQ)
        
        # ================= stage V =================
        # V[t] [s2-tile=128, 4*128] = hs @ Wv + bv (natural layout), bf16.
        for sweep in range(2):
            ts = range(8 * sweep, 8 * sweep + 8)
            vbank = {
                t: psK.tile([P, NKV * D], F32, tag="pj", name=f"pv_{t}")
                for t in ts
            }
            for ht in range(NT):
                wvt = s1p.tile([P, NKV * D], BF16, tag="wv", bufs=3, name="wvt")
                nc.sync.dma_start(
                    wvt[:],
                    mega[MG_WKV + ht * P : MG_WKV + (ht + 1) * P,
                         NKV * D : 2 * NKV * D],
                )
                for t in ts:
                    nc.tensor.matmul(
                        vbank[t][:],
                        hs[ht][:, t * P : (t + 1) * P],
                        wvt[:],
                        start=(ht == 0),
                        stop=False,
                    )
            for t in ts:
                nc.tensor.matmul(
                    vbank[t][:], ones1b[:], bvb_sb[:], start=False, stop=True
                )
                nc.scalar.copy(vt[t][:], vbank[t][:])
        s1p.release()
        hsp.release()
        tbl.release()
        psK.release()

        # ================= phase 2: attention =================
        # wso is allocated before asp (deeper on the right stack) so the
        # output-projection weight prefetch can run on the idle SP ring
        # during attention.
        wso = tc.alloc_tile_pool(name="wso", bufs=4, side="right")
        pa = tc.alloc_tile_pool(name="ps_a", bufs=5, space="PSUM")
        pc = tc.alloc_tile_pool(name="ps_c", bufs=3, space="PSUM")
        asp = tc.alloc_tile_pool(name="asp", bufs=2, side="right")
        NTT = S // P  # 16 key tiles

        for h in range(NH):
            g = h // (NH // NKV)
            ctx_ps = pc.tile([P, SQ], F32, tag="acc", name=f"ctxps{h}")
            den_ps = pc.tile([P, SQ], F32, tag="acc", name=f"denps{h}")
            at = {}

            def emit_sc(t):
                sc = pa.tile([P, SQ], F32, tag="sc", bufs=5, name="sc")
                nc.tensor.matmul(
                    sc[:],
                    kt[g][:, t * P : (t + 1) * P],
                    qt[h][:],
                    start=True,
                    stop=True,
                )
                a = asp.tile([P, SQ], BF16, tag="at", bufs=5, name="at")
                nc.scalar.activation(a[:], sc[:], AF.Exp, scale=SCALE)
                at[t] = a

            emit_sc(0)
            emit_sc(1)
            for t in range(NTT):
                if t + 2 < NTT:
                    emit_sc(t + 2)
                nc.tensor.matmul(
                    ctx_ps[:],
                    vt[t][:, g * D : (g + 1) * D],
                    at[t][:],
                    start=(t == 0),
                    stop=(t == NTT - 1),
                )
                nc.tensor.matmul(
                    den_ps[:],
                    ones128b[:],
                    at[t][:],
                    start=(t == 0),
                    stop=(t == NTT - 1),
                )
                del at[t]
            rc = asp.tile([P, SQ], F32, tag="rc", bufs=2, name="rc")
            nc.vector.reciprocal(rc[:], den_ps[:])
            nc.vector.tensor_mul(ctx[h][:], ctx_ps[:], rc[:])
        asp.release()
        pc.release()
        pa.release()

        # ================= phase 3: output projection =================
        # s1t-major: each PSUM bank finishes all 16 head matmuls before the
        # next bank starts, so its copy+store overlaps the next bank's
        # matmuls instead of piling up at the end. The per-chunk Wo tiles
        # are prefetched (SP is idle during attention).
        po = tc.alloc_tile_pool(name="ps_o", bufs=8, space="PSUM")
        for hc in range(4):
            wots = []
            for h in range(NH):
                wot = wso.tile([P, 512], BF16, tag="wo", bufs=18, name="wot")
                nc.sync.dma_start(
                    wot[:],
                    mega[MG_WO + h * D : MG_WO + (h + 1) * D,
                         hc * 512 : (hc + 1) * 512],
                )
                wots.append(wot)
            banks = [
                po.tile([P, 512], F32, tag="po", name=f"po_{hc}_{i}")
                for i in range(4)
            ]
            for s1t in range(4):
                for h in range(NH):
                    nc.tensor.matmul(
                        banks[s1t][:],
                        ctx[h][:, s1t * P : (s1t + 1) * P],
                        wots[h][:],
                        start=(h == 0),
                        stop=(h == NH - 1),
                    )
                ob = wso.tile([P, 512], F32, tag="ob", bufs=4, name="ob")
                nc.scalar.copy(ob[:], banks[s1t][:])
                nc.sync.dma_start(
                    out[s1t * P : (s1t + 1) * P, hc * 512 : (hc + 1) * 512],
                    ob[:],
                )
        wso.release()
        po.release()
        big.release()
        cst.release()

    nc.compile()
    return nc


_PROGRAM_CACHE = {}


def _get_program():
    if "nc" not in _PROGRAM_CACHE:
        _PROGRAM_CACHE["nc"] = _build_program()
    return _PROGRAM_CACHE["nc"]


def _prepare_in_maps(hidden_states, Wq, bq, Wk, bk, Wv, bv, Wo):
    import ml_dtypes

    BF = ml_dtypes.bfloat16

    hidden_states = np.asarray(hidden_states, dtype=np.float32)
    Wq = np.asarray(Wq, dtype=np.float32)
    bq = np.asarray(bq, dtype=np.float32)
    Wk = np.asarray(Wk, dtype=np.float32)
    bk = np.asarray(bk, dtype=np.float32)
    Wv = np.asarray(Wv, dtype=np.float32)
    bv = np.asarray(bv, dtype=np.float32)
    Wo = np.asarray(Wo, dtype=np.float32)

    cosT, ssinT = _rope_tables_T()
    bqk_h = np.concatenate(
        [bq.reshape(NH, D).T, bk.reshape(NKV, D).T], axis=1
    )  # [128, 20]

    # weight block of `mega` (rows H:) is identical for every core
    wblock = np.empty((3 * H + 1, H), dtype=BF)
    wblock[0:H] = Wq.astype(BF)
    wblock[H : 2 * H] = Wo.astype(BF)
    wblock[2 * H :] = 0
    wblock[2 * H : 3 * H, : NKV * D] = Wk.astype(BF)
    wblock[2 * H : 3 * H, NKV * D : 2 * NKV * D] = Wv.astype(BF)
    wblock[3 * H, NKV * D : 2 * NKV * D] = bv.astype(BF)

    hsT_b = [np.ascontiguousarray(hidden_states[b].T).astype(BF)
             for b in range(B)]

    def roll(a, qoff):
        if qoff == 0:
            return a
        return np.concatenate([a[:, qoff:], a[:, :qoff]], axis=1)

    in_maps = []
    for core in range(NCORES):
        b, tq = core // 4, core % 4
        qoff = tq * SQ
        mega_h = np.empty((4 * H + 1, H), dtype=BF)
        mega_h[0:H] = roll(hsT_b[b], qoff)
        mega_h[H:] = wblock
        cos_r, ssin_r = roll(cosT, qoff), roll(ssinT, qoff)
        misc_h = np.concatenate(
            [bqk_h, cos_r, ssin_r, cos_r[:, :SQ], ssin_r[:, :SQ]], axis=1
        )
        in_maps.append(
            {
                "mega": mega_h,
                "misc": np.ascontiguousarray(misc_h),
            }
        )
    return in_maps


def kernel(hidden_states, Wq, bq, Wk, bk, Wv, bv, Wo):
    from concourse.bass_utils import run_bass_kernel_spmd

    in_maps = _prepare_in_maps(hidden_states, Wq, bq, Wk, bk, Wv, bv, Wo)
    nc = _get_program()
    res = run_bass_kernel_spmd(
        nc, in_maps, core_ids=list(range(NCORES)), trace=False
    )

    out_full = np.empty((B, S, H), dtype=np.float32)
    for core in range(NCORES):
        b, tq = core // 4, core % 4
        out_full[b, tq * SQ : (tq + 1) * SQ, :] = res.results[core]["out"]
    return out_full
